# revision 14
# baseline (speedup 1.0000x reference)
"""CantorAttention Trainium2 kernel (8 NeuronCores, SPMD) — v2.

Strategy (same sharding as v1):
  - Shard batch (2) x head-groups (4 heads each) across the 8 cores.
  - Host: sort sequence positions by Cantor value so every 128-query block
    attends to a contiguous band of <=256 keys; count-masks pick the routed
    keys.
  - Device: QKV projection, banded masked attention, output projection.
  - Host: sum per-batch partials, add b_out, un-permute.

v2 program (fast path, nb==2 everywhere):
  - Inputs land via contiguous-per-partition DMA pieces spread over 4 queues
    so the first projection matmul fires at ~2us.
  - Attention PV matmul is computed TRANSPOSED (stationary = exp weights,
    moving = v-band): output is [query-partition, head, 64v + denominator].
    The softmax denominator is then a per-partition scalar: one [128,4]
    fast-reciprocal + one broadcast multiply normalizes - no DRAM round
    trip, no staging copies.
  - Normalized attention is transposed back to [feature, query] through the
    PE (identity-transpose matmul) for the output projection.
  - Merged software pipeline: per step s: v-proj(s+2), scores(s), exp(s),
    mask(s-1), PV(s-2), recip/norm(s-3), transpose(s-4), out-proj(s-5),
    psum->sbuf copies + output DMA (s-6). Engine queues are ordered so the
    head of every queue is always ready.
"""

import os
import sys

sys.path.insert(0, "/opt/trn_rl_repo")

import numpy as np
import ml_dtypes

import concourse.bass as bass
import concourse.mybir as mybir
import concourse.tile as tile
from concourse import bacc
from concourse.bass_utils import run_bass_kernel_spmd

B, S, DIM, H, HD, KNN, DEPTH = 2, 2048, 1024, 16, 64, 64, 8
SCALE = 1.0 / np.sqrt(HD)
N_CORES = 8
HPC = H // (N_CORES // B)       # heads per core = 4
FQK = 2 * HPC * HD              # q+k rows per core = 512
BLK = 128                       # queries per attention block
NBLK = S // BLK                 # 16
KT = DIM // 128                 # 8 contraction tiles

F32 = mybir.dt.float32
BF16 = mybir.dt.bfloat16
BF16NP = ml_dtypes.bfloat16

LAST_RESULTS = None  # BassKernelResults of the most recent run (for test.py)
_PROGRAM_CACHE = {}


def _ensure_axon_hooks():
    """Provide antenv.axon_hooks if the image lacks it, wiring the NTFF
    profile hook from the boot shim so BASS_TRACE=1 can capture timings."""
    try:
        import antenv.axon_hooks  # noqa: F401
        return
    except ImportError:
        pass
    import types
    import antenv
    hook = None
    try:
        from trn_agent_boot.trn_boot import _ntff_profile_via_ctypes
        if os.path.exists("/opt/axon/libaxon_pjrt.so"):
            hook = _ntff_profile_via_ctypes("/opt/axon/libaxon_pjrt.so")
    except Exception:
        hook = None
    mod = types.ModuleType("antenv.axon_hooks")
    mod.get_axon_ntff_profile_hook = lambda: hook
    mod.set_axon_ntff_profile_hook = lambda h: None
    sys.modules["antenv.axon_hooks"] = mod
    antenv.axon_hooks = mod


def _patch_upload():
    """Don't attempt S3 artifact uploads from the sandbox."""
    import concourse.bass_utils as bu
    bu.upload_artifacts = lambda tmpdir: str(tmpdir)


_ensure_axon_hooks()
_patch_upload()


def _cantor_values(seq_len, depth):
    pos = np.arange(seq_len, dtype=np.float64)
    x = pos / max(1, seq_len - 1)
    x = np.clip(x, 1e-06, 1.0 - 1e-06)
    cantor = np.zeros(seq_len, dtype=np.float64)
    factor = 0.5
    for _ in range(depth):
        x = x * 3.0
        digit = np.floor(x)
        x = x - digit
        cantor += factor * (digit == 2.0)
        factor *= 0.5
    return cantor.astype(np.float32)


def _plan_bands(routes_p):
    """Per 128-query block: (lo, n_subtiles) with 128-multiple band widths."""
    lo_all = routes_p.min(axis=1).reshape(NBLK, BLK).min(axis=1)
    hi_all = (routes_p.max(axis=1) + 1).reshape(NBLK, BLK).max(axis=1)
    bands = []
    for b in range(NBLK):
        lo, hi = int(lo_all[b]), int(hi_all[b])
        lo = (lo // 32) * 32       # engine ops need 32-aligned start partitions
        u = int(np.ceil((hi - lo) / 128.0)) * 128
        u = max(u, 128)
        lo = min(lo, S - u)
        bands.append((lo, u // 128))
    return bands


def _build_masks(routes_p, bands, dup):
    """Count-masks in device layout [128, nU, dup, BLK] bf16."""
    parts = []
    for b, (lo, nb) in enumerate(bands):
        rel = routes_p[b * BLK:(b + 1) * BLK] - lo          # [BLK, KNN]
        m = np.zeros((nb * 128, BLK), dtype=np.float32)
        qidx = np.broadcast_to(np.arange(BLK)[:, None], rel.shape)
        np.add.at(m, (rel, qidx), 1.0)
        parts.append(m)
    mk = np.concatenate(parts, axis=0)                      # [nU*128, BLK]
    nU = mk.shape[0] // 128
    mk = mk.reshape(nU, 128, BLK).transpose(1, 0, 2)        # [128, nU, BLK]
    if dup == 1:
        return np.ascontiguousarray(mk).astype(BF16NP)
    mk = np.broadcast_to(mk[:, :, None], (128, nU, dup, BLK))
    return np.ascontiguousarray(mk).astype(BF16NP)


def _build_program_fast(bands):
    """v2 software-pipelined SPMD program for band plans with nb == 2."""
    assert all(nb == 2 for _, nb in bands)
    nU = 2 * NBLK

    nc = bacc.Bacc("TRN2", target_bir_lowering=False)

    # DRAM inputs, pre-arranged so each DMA piece is one contiguous run per
    # partition (trivial descriptors):
    #   xTp  [128, 8 pieces, 8 kt, 256]   x^T in 256-seq pieces
    #   wqp  [128, 3 (q|k|v), 8 kt, 256]  weight column groups
    xT_d = nc.dram_tensor("xTp", [128, 8, KT, 256], BF16, kind="ExternalInput")
    wq_d = nc.dram_tensor("wqp", [128, 3, KT, 256], BF16, kind="ExternalInput")
    bqk_d = nc.dram_tensor("bqkp", [128, FQK // 128], F32, kind="ExternalInput")
    bv_d = nc.dram_tensor("bv", [HPC * HD], F32, kind="ExternalInput")
    wo_d = nc.dram_tensor("wop", [128, 2, DIM], BF16, kind="ExternalInput")
    mask_d = nc.dram_tensor("maskT", [128, nU, BLK], BF16, kind="ExternalInput")
    id_d = nc.dram_tensor("ident", [128, 128], BF16, kind="ExternalInput")
    out_d = nc.dram_tensor("out_p", [S, DIM], BF16, kind="ExternalOutput")

    # per-block step at which its v-band chunks are all projected
    ready_chunk = []
    for lo, _ in bands:
        a0, sp = lo // 128, lo % 128
        ready_chunk.append(a0 + (2 if sp else 1))
    vpk_at = {}
    for b in range(NBLK):
        st = max(b - 1, ready_chunk[b] - 2, -1)
        vpk_at.setdefault(st, []).append(b)

    with tile.TileContext(nc) as tc:
        with tc.tile_pool(name="const", bufs=1) as cpool, \
             tc.tile_pool(name="work", bufs=1) as wpool, \
             tc.tile_pool(name="epool", bufs=4) as epool, \
             tc.tile_pool(name="aq", bufs=2) as aqpool, \
             tc.tile_pool(name="at", bufs=3) as atpool, \
             tc.tile_pool(name="rc", bufs=2) as rcpool, \
             tc.tile_pool(name="ob", bufs=2) as obpool, \
             tc.tile_pool(name="pst", bufs=1, space="PSUM") as pstp, \
             tc.tile_pool(name="pvq", bufs=2, space="PSUM") as pvqp, \
             tc.tile_pool(name="ptr", bufs=1, space="PSUM") as ptrp, \
             tc.tile_pool(name="pb", bufs=1, space="PSUM") as pbp, \
             tc.tile_pool(name="pp", bufs=1, space="PSUM") as pop:

            # ---- input DMAs: contiguous pieces over 4 queues ----
            xT = cpool.tile([128, 8, KT, 256], BF16, tag="xT")
            wq = cpool.tile([128, 3, KT, 256], BF16, tag="wq")
            bqk = cpool.tile([128, FQK // 128], F32, tag="bqk")
            bvb = cpool.tile([128, HPC, HD], F32, tag="bvb")
            wo = cpool.tile([128, 2, DIM], BF16, tag="wo")
            mk = cpool.tile([128, nU, BLK], BF16, tag="mask")
            ident = cpool.tile([128, 128], BF16, tag="ident")

            # Both HWDGE queues (sync/scalar) run ~100-185 GB/s each and
            # nothing issues before the ~7us framework preamble. Tiny consts
            # first, then pieces in exact consumption order, balanced across
            # the two queues; mask split so block 0's slice lands early.
            # gpsimd (SWDGE, slow start) gets nothing critical.
            nc.sync.dma_start(bqk[:], bqk_d[:])
            nc.sync.dma_start(wq[:, 0], wq_d[:, 0])         # q cols
            nc.scalar.dma_start(xT[:, 1], xT_d[:, 1])
            nc.sync.dma_start(xT[:, 0], xT_d[:, 0])
            nc.scalar.dma_start(wq[:, 1], wq_d[:, 1])       # k cols
            nc.sync.dma_start(wq[:, 2], wq_d[:, 2])         # v cols
            nc.scalar.dma_start(
                bvb[:], bv_d.rearrange("(h d) -> h d", h=HPC)[None, :, :]
                .to_broadcast((128, HPC, HD)))
            nc.scalar.dma_start(ident[:], id_d[:])
            nc.scalar.dma_start(xT[:, 3], xT_d[:, 3])
            nc.sync.dma_start(xT[:, 2], xT_d[:, 2])
            nc.sync.dma_start(mk[:, :8], mask_d[:, :8])
            nc.scalar.dma_start(xT[:, 6:8], xT_d[:, 6:8])
            nc.sync.dma_start(xT[:, 4:6], xT_d[:, 4:6])
            nc.sync.dma_start(mk[:, 8:], mask_d[:, 8:])
            nc.scalar.dma_start(wo[:], wo_d[:])

            # ---- phase A: q/k projection -> qk_sb [128, 4, S] ----
            # ft 0,1 = q for head-pair col groups hp0/hp1; ft 2,3 = k.
            # partitions: p = 64*(h%2) + d within each ft tile.
            # Emitted interleaved with the merged pipeline (emit_A below).
            qk_sb = wpool.tile([128, FQK // 128, S], BF16, tag="qk")

            def emit_A(pc):
                # q/k projection for one 256-seq piece (starts on one x DMA)
                for ft in range(4):
                    pt = pop.tile([128, 512], F32, tag=f"po{ft % 2}",
                                  bufs=1, name=f"po{ft % 2}")
                    for kt in range(KT):
                        nc.tensor.matmul(
                            pt[:, :256],
                            wq[:, ft // 2, kt,
                               (ft % 2) * 128:(ft % 2) * 128 + 128],
                            xT[:, pc, kt, :],
                            start=(kt == 0), stop=(kt == KT - 1))
                    nc.scalar.activation(
                        qk_sb[:, ft, pc * 256:(pc + 1) * 256], pt[:, :256],
                        mybir.ActivationFunctionType.Identity,
                        bias=bqk[:, ft:ft + 1])

            # ---- persistent attention tiles ----
            v_sb = wpool.tile([128, S // 128, HPC, HD + 1], BF16, tag="v")
            nc.vector.memset(v_sb[:, :, :, HD:HD + 1], 1.0)
            vpk = wpool.tile([128, nU, HPC, HD + 1], BF16, tag="vpk")

            def issue_vpk(b):
                lo, _ = bands[b]
                a0, sp = lo // 128, lo % 128
                if sp == 0:
                    nc.gpsimd.dma_start(vpk[:, 2 * b:2 * b + 2], v_sb[:, a0:a0 + 2])
                else:
                    nc.gpsimd.dma_start(vpk[:128 - sp, 2 * b:2 * b + 2],
                                        v_sb[sp:, a0:a0 + 2])
                    nc.gpsimd.dma_start(vpk[128 - sp:, 2 * b:2 * b + 2],
                                        v_sb[:sp, a0 + 1:a0 + 3])

            emts = [None] * NBLK
            pvqs = [None] * NBLK
            aqs = [None] * NBLK
            ats = [None] * NBLK
            pos_ = [None] * NBLK
            obts = [None] * NBLK

            # ---- merged pipeline (phase A interleaved per 256-piece) ----
            emit_A(0)
            emit_A(1)
            for s in range(-2, NBLK + 5):

                # v projection chunk s+2 (PE) + bias add (DVE)
                c = s + 2
                if 0 <= c < NBLK:
                    pb = pbp.tile([128, 512], F32, tag="pb", name="pb")
                    for kt in range(KT):
                        nc.tensor.matmul(
                            pb[:, :256],
                            xT[:, c // 2, kt, (c % 2) * 128:(c % 2) * 128 + 128],
                            wq[:, 2, kt, :],
                            start=(kt == 0), stop=(kt == KT - 1))
                    nc.vector.tensor_add(
                        v_sb[:, c, :, :HD],
                        pb[:, :256].rearrange("p (h d) -> p h d", h=HPC),
                        bvb[:])
                for b in vpk_at.get(s, ()):
                    issue_vpk(b)

                # psum->sbuf output copies + DMA for block s-4
                b5 = s - 4
                if 0 <= b5 < NBLK:
                    obt = obpool.tile([128, DIM], BF16, tag="obt")
                    nc.scalar.copy(obt[:, :512], pos_[b5][0][:])
                    nc.sync.dma_start(
                        out_d[b5 * BLK:(b5 + 1) * BLK, :512], obt[:, :512])
                    nc.vector.tensor_copy(obt[:, 512:], pos_[b5][1][:])
                    obts[b5] = obt
                    nc.sync.dma_start(
                        out_d[b5 * BLK:(b5 + 1) * BLK, 512:], obt[:, 512:])

                # scores for block s: psum [128k, hh, 2*iu+hp, q]
                if 0 <= s < NBLK:
                    lo, _ = bands[s]
                    qs = slice(s * BLK, (s + 1) * BLK)
                    pst = pstp.tile([128, 2, HPC, BLK], F32, tag="pst")
                    for iu in range(2):
                        for hp in range(2):
                            for hh in range(2):
                                nc.tensor.matmul(
                                    pst[:, hh, 2 * iu + hp, :],
                                    qk_sb[64 * hh:64 * hh + 64, 2 + hp,
                                          lo + iu * 128: lo + (iu + 1) * 128],
                                    qk_sb[64 * hh:64 * hh + 64, hp, qs],
                                    start=True, stop=True)
                    et = epool.tile([128, 2, HPC, BLK], BF16, tag="et")
                    nc.scalar.activation(
                        et[:], pst[:], mybir.ActivationFunctionType.Exp,
                        scale=float(SCALE))
                    emts[s] = et  # masked in next step (emt reuses slot)

                # mask multiply for block s (DVE + Pool split)
                b1 = s
                if 0 <= b1 < NBLK:
                    et = emts[b1]
                    emt = epool.tile([128, 2, HPC, BLK], BF16, tag="emt")
                    for iu, eng in ((0, nc.vector), (1, nc.gpsimd)):
                        eng.tensor_mul(
                            emt[:, :, 2 * iu:2 * iu + 2, :],
                            et[:, :, 2 * iu:2 * iu + 2, :],
                            mk[:, 2 * b1 + iu, None, None, :]
                            .broadcast_to((128, 2, 2, BLK)))
                    emts[b1] = emt

                # PV transposed for block s-1: out [q, head, 64v + den],
                # then reciprocal + normalize (DVE, same step)
                b2 = s - 1
                if 0 <= b2 < NBLK:
                    emt = emts[b2]
                    pvq = pvqp.tile([128, HPC, 128], F32, tag="pvq")
                    for h in range(HPC):
                        for iu in range(2):
                            nc.tensor.matmul(
                                pvq[:, h, :HD + 1],
                                emt[:, h % 2, 2 * iu + h // 2, :],
                                vpk[:, 2 * b2 + iu, h, :],
                                start=(iu == 0), stop=(iu == 1))
                    rec = rcpool.tile([128, HPC, 1], F32, tag="rec")
                    nc.vector.reciprocal_approx_fast(
                        out=rec[:], in_=pvq[:, :, HD:HD + 1])
                    aq = aqpool.tile([128, HPC, HD], BF16, tag="aq")
                    nc.vector.tensor_mul(
                        aq[:], pvq[:, :, :HD],
                        rec[:].broadcast_to((128, HPC, HD)))
                    aqs[b2] = aq

                # transpose back to [feature, q] for block s-2 (PE + DVE copy)
                b3 = s - 2
                if 0 <= b3 < NBLK:
                    aq = aqs[b3]
                    ptr = ptrp.tile([128, 2, 512], BF16, tag="ptr", name="ptr")
                    for dt in range(2):
                        nc.tensor.transpose(
                            ptr[:, dt, :128],
                            aq[:, 2 * dt:2 * dt + 2, :].rearrange(
                                "p a b -> p (a b)"),
                            ident[:])
                    at = atpool.tile([128, 2, BLK], BF16, tag="at")
                    nc.vector.tensor_copy(at[:], ptr[:, :, :128])
                    ats[b3] = at

                # output projection for block s-3 (PE)
                b4 = s - 3
                if 0 <= b4 < NBLK:
                    at = ats[b4]
                    pts = []
                    for ot in range(2):
                        pt = pop.tile([128, 512], F32, tag=f"po{ot}",
                                      bufs=1, name=f"po{ot}")
                        for dt in range(2):
                            nc.tensor.matmul(
                                pt[:],
                                at[:, dt, :],
                                wo[:, dt, ot * 512:(ot + 1) * 512],
                                start=(dt == 0), stop=(dt == 1))
                        pts.append(pt)
                    pos_[b4] = pts

                # phase-A piece rides the step tail (PE slack filler);
                # scores(s) never queues behind act-gated A matmuls.
                if 0 <= s <= 10 and s % 2 == 0:
                    emit_A(2 + s // 2)

    nc.finalize()
    return nc


def _build_program_generic(bands):
    """Original (slower) program: handles arbitrary band widths."""
    nU = sum(nb for _, nb in bands)
    nb_max = max(nb for _, nb in bands)

    nc = bacc.Bacc("TRN2", target_bir_lowering=False)

    xT_d = nc.dram_tensor("xT", [DIM, S], BF16, kind="ExternalInput")
    wq_d = nc.dram_tensor("wqkvT", [DIM, FQK + HPC * HD], BF16, kind="ExternalInput")
    bqk_d = nc.dram_tensor("bqkp", [128, FQK // 128], F32, kind="ExternalInput")
    bv_d = nc.dram_tensor("bv", [HPC * HD], F32, kind="ExternalInput")
    wo_d = nc.dram_tensor("woT", [HPC * HD, DIM], BF16, kind="ExternalInput")
    # pre-arranged mask layout [128, nU, 2, BLK] (head-pair duplicated), bf16
    mask_d = nc.dram_tensor("maskT", [128, nU, 2, BLK], BF16, kind="ExternalInput")
    out_d = nc.dram_tensor("out_p", [S, DIM], BF16, kind="ExternalOutput")

    with tile.TileContext(nc) as tc:
        with tc.tile_pool(name="const", bufs=1) as cpool, \
             tc.tile_pool(name="work", bufs=1) as wpool, \
             tc.tile_pool(name="epool", bufs=6) as epool, \
             tc.tile_pool(name="spool", bufs=2) as spool, \
             tc.tile_pool(name="dram", bufs=1, space="DRAM") as dpool, \
             tc.tile_pool(name="pp", bufs=2, space="PSUM") as pp, \
             tc.tile_pool(name="ps", bufs=3, space="PSUM") as ps, \
             tc.tile_pool(name="pv", bufs=3, space="PSUM") as pv:

            # ---- constant loads ----
            xT = cpool.tile([128, KT, S], BF16, tag="xT")
            for kt in range(KT):
                nc.sync.dma_start(
                    xT[:, kt, :],
                    xT_d.rearrange("(t p) s -> p t s", p=128)[:, kt, :])
            wq = cpool.tile([128, KT, FQK + HPC * HD], BF16, tag="wq")
            nc.sync.dma_start(wq[:], wq_d.rearrange("(t p) f -> p t f", p=128))
            bqk = cpool.tile([128, FQK // 128], F32, tag="bqk")
            nc.sync.dma_start(bqk[:], bqk_d[:])
            bvb = cpool.tile([128, HPC * HD], F32, tag="bvb")
            nc.sync.dma_start(bvb[:], bv_d[None, :].to_broadcast((128, HPC * HD)))
            wo = cpool.tile([128, 2, DIM], BF16, tag="wo")
            nc.sync.dma_start(wo[:], wo_d.rearrange("(t p) o -> p t o", p=128))

            # ---- phase A: q/k projection -> qk_sb [128, 4, S] (f-major) ----
            qk_sb = wpool.tile([128, FQK // 128, S], BF16, tag="qk")
            for ft in range(FQK // 128):
                for st in range(S // 512):
                    pt = pp.tile([128, 512], F32, tag="pp")
                    for kt in range(KT):
                        nc.tensor.matmul(
                            pt[:],
                            wq[:, kt, ft * 128:(ft + 1) * 128],
                            xT[:, kt, st * 512:(st + 1) * 512],
                            start=(kt == 0), stop=(kt == KT - 1))
                    nc.scalar.activation(
                        qk_sb[:, ft, st * 512:(st + 1) * 512], pt[:],
                        mybir.ActivationFunctionType.Identity,
                        bias=bqk[:, ft:ft + 1])

            # ---- phase B: v projection -> v_sb [128, 16, HPC, 65] ----
            v_sb = wpool.tile([128, S // 128, HPC, HD + 1], BF16, tag="v")
            nc.vector.memset(v_sb[:], 1.0)
            for st in range(S // 128):
                pt = pp.tile([128, 512], F32, tag="pp")
                for kt in range(KT):
                    nc.tensor.matmul(
                        pt[:, :HPC * HD],
                        xT[:, kt, st * 128:(st + 1) * 128],
                        wq[:, kt, FQK:],
                        start=(kt == 0), stop=(kt == KT - 1))
                nc.vector.tensor_add(
                    v_sb[:, st, :, :HD],
                    pt[:, :HPC * HD].rearrange("p (h d) -> p h d", h=HPC),
                    bvb.rearrange("p (h d) -> p h d", h=HPC))

            # ---- attention ----
            stg_un = wpool.tile([128, 2, S], F32, tag="stg")     # unnormalized attnT
            den_dram = dpool.tile([HPC, S], F32)
            den_sb = [wpool.tile([1, S], F32, tag=f"den{h}", name=f"den_sb{h}")
                      for h in range(HPC)]

            def pack_band(dst, dsl, b):
                """band-pack v (+ones cols) via DVE cross-base chunk copies."""
                lo, nb = bands[b]
                a0, r = lo // 128, lo % 128
                if r == 0:
                    nc.vector.tensor_copy(dst[:, dsl], v_sb[:, a0:a0 + nb])
                else:
                    for j in range(4):
                        sp = (r + 32 * j) % 128
                        sa = a0 + (1 if r + 32 * j >= 128 else 0)
                        nc.vector.tensor_copy(
                            dst[32 * j:32 * (j + 1), dsl],
                            v_sb[sp:sp + 32, sa:sa + nb])

            moff = 0
            for b in range(NBLK):
                lo, nb = bands[b]
                qs = slice(b * BLK, (b + 1) * BLK)
                mkb = spool.tile([128, nb_max, 2, BLK], BF16, tag="mkb")
                nc.sync.dma_start(mkb[:, :nb], mask_d[:, moff:moff + nb])
                vpb = spool.tile([128, nb_max, HPC, HD + 1], BF16, tag="vpb")
                pack_band(vpb, slice(0, nb), b)
                for h in range(HPC):
                    hh, hp = h % 2, h // 2
                    pvt = pv.tile([HD + 1, BLK], F32, tag="pv")
                    for iu in range(nb):
                        pst = ps.tile([128, BLK], F32, tag="ps")
                        nc.tensor.matmul(
                            pst[:],
                            qk_sb[64 * hh:64 * hh + 64, 2 + hp,
                                  lo + iu * 128: lo + (iu + 1) * 128],
                            qk_sb[64 * hh:64 * hh + 64, hp, qs],
                            start=True, stop=True)
                        et = epool.tile([128, BLK], BF16, tag="e")
                        nc.scalar.activation(
                            et[:], pst[:], mybir.ActivationFunctionType.Exp,
                            scale=float(SCALE))
                        emt = epool.tile([128, BLK], BF16, tag="em")
                        nc.vector.tensor_mul(emt[:], et[:],
                                             mkb[:, iu, hh, :])
                        nc.tensor.matmul(
                            pvt[:], vpb[:, iu, h, :], emt[:],
                            start=(iu == 0), stop=(iu == nb - 1))
                    nc.scalar.copy(stg_un[64 * hh:64 * hh + 64, hp, qs], pvt[:HD, :])
                    nc.vector.tensor_copy(den_sb[h][0:1, qs], pvt[HD:HD + 1, :])
                moff += nb

            # ---- normalize + output projection, pipelined in s-quarters ----
            rec_dram = dpool.tile([HPC, S], F32)
            denr = wpool.tile([128, HPC, S // 128], F32, tag="denr")
            recr = wpool.tile([128, HPC, S // 128], F32, tag="recr")
            rec_bc = wpool.tile([128, 2, S], F32, tag="denbc")
            attnT = wpool.tile([128, 2, S], BF16, tag="attnT")
            NQ = 4
            SQ = S // NQ
            AQ = SQ // 128
            for q in range(NQ):
                sq = slice(q * SQ, (q + 1) * SQ)
                for h in range(HPC):
                    nc.sync.dma_start(den_dram[h:h + 1, sq], den_sb[h][0:1, sq])
                nc.sync.dma_start(
                    denr[:, :, q * AQ:(q + 1) * AQ],
                    den_dram[:, sq].rearrange("h (p a) -> p h a", p=128))
                nc.vector.reciprocal(recr[:, :, q * AQ:(q + 1) * AQ],
                                     denr[:, :, q * AQ:(q + 1) * AQ])
                nc.sync.dma_start(
                    rec_dram[:, sq].rearrange("h (p a) -> p h a", p=128),
                    recr[:, :, q * AQ:(q + 1) * AQ])
                for dt in range(2):
                    for hh in range(2):
                        h = 2 * dt + hh
                        nc.sync.dma_start(
                            rec_bc[64 * hh:64 * (hh + 1), dt, sq],
                            rec_dram[h:h + 1, sq].to_broadcast((64, SQ)))
                    nc.vector.tensor_mul(
                        attnT[:, dt, sq], stg_un[:, dt, sq], rec_bc[:, dt, sq])
                for st in range(q * (S // 128) // NQ, (q + 1) * (S // 128) // NQ):
                    for ot in range(DIM // 512):
                        po = pp.tile([128, 512], F32, tag="pp")
                        for dt in range(2):
                            nc.tensor.matmul(
                                po[:],
                                attnT[:, dt, st * 128:(st + 1) * 128],
                                wo[:, dt, ot * 512:(ot + 1) * 512],
                                start=(dt == 0), stop=(dt == 1))
                        ob = epool.tile([128, 512], BF16, tag="ob")
                        nc.vector.tensor_copy(ob[:], po[:])
                        nc.sync.dma_start(
                            out_d[st * 128:(st + 1) * 128, ot * 512:(ot + 1) * 512],
                            ob[:])

    nc.finalize()
    return nc


def kernel(x, w_qkv, b_qkv, w_out, b_out, routes):
    global LAST_RESULTS
    x = np.asarray(x, dtype=np.float32)
    w_qkv = np.asarray(w_qkv, dtype=np.float32)
    b_qkv = np.asarray(b_qkv, dtype=np.float32)
    w_out = np.asarray(w_out, dtype=np.float32)
    b_out = np.asarray(b_out, dtype=np.float32)
    routes = np.asarray(routes)

    # --- host: permutation + bands + masks ---
    cantor = _cantor_values(S, DEPTH)
    perm = np.lexsort((np.arange(S), cantor))
    inv_perm = np.empty(S, dtype=np.int64)
    inv_perm[perm] = np.arange(S)
    routes_p = inv_perm[routes.astype(np.int64)[perm]]
    bands = _plan_bands(routes_p)
    fast = all(nb == 2 for _, nb in bands)
    maskT = _build_masks(routes_p, bands, 1 if fast else 2)

    key = (tuple(bands), "v2")
    if key not in _PROGRAM_CACHE:
        _PROGRAM_CACHE[key] = (
            _build_program_fast(bands) if fast else _build_program_generic(bands))
    nc = _PROGRAM_CACHE[key]

    # --- host: per-core inputs ---
    x_p = x[:, perm, :]                                   # [B, S, DIM]
    ident = np.eye(128, dtype=BF16NP)
    in_maps = []
    for c in range(N_CORES):
        b = c // (N_CORES // B)
        hg = c % (N_CORES // B)
        heads = range(hg * HPC, (hg + 1) * HPC)
        # w rows: q heads, k heads, v heads
        rows = ([h * HD + i for h in heads for i in range(HD)]
                + [DIM + h * HD + i for h in heads for i in range(HD)]
                + [2 * DIM + h * HD + i for h in heads for i in range(HD)])
        rows = np.asarray(rows)
        wq_c = np.ascontiguousarray(w_qkv[rows].T).astype(BF16NP)   # [1024, 768]
        # bias packed [128, 4] partition-major (feature = ft*128 + p)
        bqk_c = np.ascontiguousarray(
            b_qkv[rows[:FQK]].reshape(FQK // 128, 128).T).astype(np.float32)
        bv_c = np.ascontiguousarray(b_qkv[rows[FQK:]]).astype(np.float32)
        wo_c = np.ascontiguousarray(
            w_out[:, hg * HPC * HD:(hg + 1) * HPC * HD].T).astype(BF16NP)
        xT_c = np.ascontiguousarray(x_p[b].T).astype(BF16NP)
        if fast:
            # piece-contiguous layouts: [128, piece/group, kt, 256]
            in_maps.append({
                "xTp": np.ascontiguousarray(
                    xT_c.reshape(KT, 128, 8, 256).transpose(1, 2, 0, 3)),
                "wqp": np.ascontiguousarray(
                    wq_c.reshape(KT, 128, 3, 256).transpose(1, 2, 0, 3)),
                "bqkp": bqk_c,
                "bv": bv_c,
                "wop": np.ascontiguousarray(
                    wo_c.reshape(2, 128, DIM).transpose(1, 0, 2)),
                "maskT": maskT,
                "ident": ident,
            })
        else:
            in_maps.append({
                "xT": xT_c,
                "wqkvT": wq_c,
                "bqkp": bqk_c,
                "bv": bv_c,
                "woT": wo_c,
                "maskT": maskT,
            })

    try:
        res = run_bass_kernel_spmd(nc, in_maps, core_ids=list(range(N_CORES)))
    except Exception:
        if os.environ.get("BASS_TRACE"):
            # tracing infra failure — retry without profiling
            os.environ["BASS_NEVER_TRACE"] = "1"
            res = run_bass_kernel_spmd(nc, in_maps, core_ids=list(range(N_CORES)))
        else:
            raise
    LAST_RESULTS = res

    out = np.zeros((B, S, DIM), dtype=np.float32)
    for c in range(N_CORES):
        out[c // (N_CORES // B)] += np.asarray(res.results[c]["out_p"],
                                               dtype=np.float32)
    out += b_out[None, None, :]
    out = out[:, inv_perm, :]    # un-permute rows
    return out


# revision 15
# speedup vs baseline: 1.0875x; 1.0875x over previous
"""CantorAttention Trainium2 kernel (8 NeuronCores, SPMD) — v2.

Strategy (same sharding as v1):
  - Shard batch (2) x head-groups (4 heads each) across the 8 cores.
  - Host: sort sequence positions by Cantor value so every 128-query block
    attends to a contiguous band of <=256 keys; count-masks pick the routed
    keys.
  - Device: QKV projection, banded masked attention, output projection.
  - Host: sum per-batch partials, add b_out, un-permute.

v2 program (fast path, nb==2 everywhere):
  - Inputs land via contiguous-per-partition DMA pieces spread over 4 queues
    so the first projection matmul fires at ~2us.
  - Attention PV matmul is computed TRANSPOSED (stationary = exp weights,
    moving = v-band): output is [query-partition, head, 64v + denominator].
    The softmax denominator is then a per-partition scalar: one [128,4]
    fast-reciprocal + one broadcast multiply normalizes - no DRAM round
    trip, no staging copies.
  - Normalized attention is transposed back to [feature, query] through the
    PE (identity-transpose matmul) for the output projection.
  - Merged software pipeline: per step s: v-proj(s+2), scores(s), exp(s),
    mask(s-1), PV(s-2), recip/norm(s-3), transpose(s-4), out-proj(s-5),
    psum->sbuf copies + output DMA (s-6). Engine queues are ordered so the
    head of every queue is always ready.
"""

import os
import sys

sys.path.insert(0, "/opt/trn_rl_repo")

import numpy as np
import ml_dtypes

import concourse.bass as bass
import concourse.mybir as mybir
import concourse.tile as tile
from concourse import bacc
from concourse.bass_utils import run_bass_kernel_spmd

B, S, DIM, H, HD, KNN, DEPTH = 2, 2048, 1024, 16, 64, 64, 8
SCALE = 1.0 / np.sqrt(HD)
N_CORES = 8
HPC = H // (N_CORES // B)       # heads per core = 4
FQK = 2 * HPC * HD              # q+k rows per core = 512
BLK = 128                       # queries per attention block
NBLK = S // BLK                 # 16
KT = DIM // 128                 # 8 contraction tiles

F32 = mybir.dt.float32
BF16 = mybir.dt.bfloat16
BF16NP = ml_dtypes.bfloat16

LAST_RESULTS = None  # BassKernelResults of the most recent run (for test.py)
_PROGRAM_CACHE = {}


def _ensure_axon_hooks():
    """Provide antenv.axon_hooks if the image lacks it, wiring the NTFF
    profile hook from the boot shim so BASS_TRACE=1 can capture timings."""
    try:
        import antenv.axon_hooks  # noqa: F401
        return
    except ImportError:
        pass
    import types
    import antenv
    hook = None
    try:
        from trn_agent_boot.trn_boot import _ntff_profile_via_ctypes
        if os.path.exists("/opt/axon/libaxon_pjrt.so"):
            hook = _ntff_profile_via_ctypes("/opt/axon/libaxon_pjrt.so")
    except Exception:
        hook = None
    mod = types.ModuleType("antenv.axon_hooks")
    mod.get_axon_ntff_profile_hook = lambda: hook
    mod.set_axon_ntff_profile_hook = lambda h: None
    sys.modules["antenv.axon_hooks"] = mod
    antenv.axon_hooks = mod


def _patch_upload():
    """Don't attempt S3 artifact uploads from the sandbox."""
    import concourse.bass_utils as bu
    bu.upload_artifacts = lambda tmpdir: str(tmpdir)


_ensure_axon_hooks()
_patch_upload()


def _cantor_values(seq_len, depth):
    pos = np.arange(seq_len, dtype=np.float64)
    x = pos / max(1, seq_len - 1)
    x = np.clip(x, 1e-06, 1.0 - 1e-06)
    cantor = np.zeros(seq_len, dtype=np.float64)
    factor = 0.5
    for _ in range(depth):
        x = x * 3.0
        digit = np.floor(x)
        x = x - digit
        cantor += factor * (digit == 2.0)
        factor *= 0.5
    return cantor.astype(np.float32)


def _plan_bands(routes_p):
    """Per 128-query block: (lo, n_subtiles) with 128-multiple band widths."""
    lo_all = routes_p.min(axis=1).reshape(NBLK, BLK).min(axis=1)
    hi_all = (routes_p.max(axis=1) + 1).reshape(NBLK, BLK).max(axis=1)
    bands = []
    for b in range(NBLK):
        lo, hi = int(lo_all[b]), int(hi_all[b])
        lo = (lo // 32) * 32       # engine ops need 32-aligned start partitions
        u = int(np.ceil((hi - lo) / 128.0)) * 128
        u = max(u, 128)
        lo = min(lo, S - u)
        bands.append((lo, u // 128))
    return bands


def _build_masks(routes_p, bands, dup):
    """Count-masks in device layout [128, nU, dup, BLK] bf16."""
    parts = []
    for b, (lo, nb) in enumerate(bands):
        rel = routes_p[b * BLK:(b + 1) * BLK] - lo          # [BLK, KNN]
        m = np.zeros((nb * 128, BLK), dtype=np.float32)
        qidx = np.broadcast_to(np.arange(BLK)[:, None], rel.shape)
        np.add.at(m, (rel, qidx), 1.0)
        parts.append(m)
    mk = np.concatenate(parts, axis=0)                      # [nU*128, BLK]
    nU = mk.shape[0] // 128
    mk = mk.reshape(nU, 128, BLK).transpose(1, 0, 2)        # [128, nU, BLK]
    if dup == 1:
        return np.ascontiguousarray(mk).astype(BF16NP)
    mk = np.broadcast_to(mk[:, :, None], (128, nU, dup, BLK))
    return np.ascontiguousarray(mk).astype(BF16NP)


def _build_program_fast(bands):
    """v2 software-pipelined SPMD program for band plans with nb == 2."""
    assert all(nb == 2 for _, nb in bands)
    nU = 2 * NBLK

    nc = bacc.Bacc("TRN2", target_bir_lowering=False)

    # DRAM inputs, pre-arranged so each DMA piece is one contiguous run per
    # partition (trivial descriptors):
    #   xTp  [128, 8 pieces, 8 kt, 256]   x^T in 256-seq pieces
    #   wqp  [128, 3 (q|k|v), 8 kt, 256]  weight column groups
    xT_d = nc.dram_tensor("xTp", [128, 8, KT, 256], BF16, kind="ExternalInput")
    wq_d = nc.dram_tensor("wqp", [128, 3, KT, 256], BF16, kind="ExternalInput")
    bqk_d = nc.dram_tensor("bqkp", [128, FQK // 128], F32, kind="ExternalInput")
    bv_d = nc.dram_tensor("bv", [HPC * HD], F32, kind="ExternalInput")
    wo_d = nc.dram_tensor("wop", [128, 2, DIM], BF16, kind="ExternalInput")
    mask_d = nc.dram_tensor("maskT", [128, nU, BLK], BF16, kind="ExternalInput")
    id_d = nc.dram_tensor("ident", [128, 128], BF16, kind="ExternalInput")
    out_d = nc.dram_tensor("out_p", [S, DIM], BF16, kind="ExternalOutput")

    # per-block step at which its v-band chunks are all projected
    ready_chunk = []
    for lo, _ in bands:
        a0, sp = lo // 128, lo % 128
        ready_chunk.append(a0 + (2 if sp else 1))
    vpk_at = {}
    for b in range(NBLK):
        st = max(b - 1, ready_chunk[b] - 2, -1)
        vpk_at.setdefault(st, []).append(b)

    with tile.TileContext(nc) as tc:
        with tc.tile_pool(name="const", bufs=1) as cpool, \
             tc.tile_pool(name="work", bufs=1) as wpool, \
             tc.tile_pool(name="epool", bufs=4) as epool, \
             tc.tile_pool(name="aq", bufs=2) as aqpool, \
             tc.tile_pool(name="at", bufs=3) as atpool, \
             tc.tile_pool(name="rc", bufs=2) as rcpool, \
             tc.tile_pool(name="ob", bufs=2) as obpool, \
             tc.tile_pool(name="pst", bufs=1, space="PSUM") as pstp, \
             tc.tile_pool(name="pvq", bufs=2, space="PSUM") as pvqp, \
             tc.tile_pool(name="ptr", bufs=1, space="PSUM") as ptrp, \
             tc.tile_pool(name="pb", bufs=1, space="PSUM") as pbp, \
             tc.tile_pool(name="pp", bufs=1, space="PSUM") as pop:

            # ---- input DMAs: contiguous pieces over 4 queues ----
            xT = cpool.tile([128, 8, KT, 256], BF16, tag="xT")
            wq = cpool.tile([128, 3, KT, 256], BF16, tag="wq")
            bqk = cpool.tile([128, FQK // 128], F32, tag="bqk")
            bvb = cpool.tile([128, HPC, HD], F32, tag="bvb")
            wo = cpool.tile([128, 2, DIM], BF16, tag="wo")
            mk = cpool.tile([128, nU, BLK], BF16, tag="mask")
            ident = cpool.tile([128, 128], BF16, tag="ident")

            # Both HWDGE queues (sync/scalar) run ~100-185 GB/s each and
            # nothing issues before the ~7us framework preamble. Tiny consts
            # first, then pieces in exact consumption order, balanced across
            # the two queues; mask split so block 0's slice lands early.
            # gpsimd (SWDGE, slow start) gets nothing critical.
            nc.sync.dma_start(bqk[:], bqk_d[:])
            nc.sync.dma_start(wq[:, 0], wq_d[:, 0])         # q cols
            nc.scalar.dma_start(xT[:, 1], xT_d[:, 1])
            nc.sync.dma_start(xT[:, 0], xT_d[:, 0])
            nc.scalar.dma_start(wq[:, 1], wq_d[:, 1])       # k cols
            nc.sync.dma_start(wq[:, 2], wq_d[:, 2])         # v cols
            nc.scalar.dma_start(
                bvb[:], bv_d.rearrange("(h d) -> h d", h=HPC)[None, :, :]
                .to_broadcast((128, HPC, HD)))
            nc.scalar.dma_start(ident[:], id_d[:])
            nc.scalar.dma_start(xT[:, 3], xT_d[:, 3])
            nc.sync.dma_start(xT[:, 2], xT_d[:, 2])
            nc.sync.dma_start(mk[:, :8], mask_d[:, :8])
            nc.scalar.dma_start(xT[:, 6:8], xT_d[:, 6:8])
            nc.sync.dma_start(xT[:, 4:6], xT_d[:, 4:6])
            nc.sync.dma_start(mk[:, 8:], mask_d[:, 8:])
            nc.scalar.dma_start(wo[:], wo_d[:])

            # ---- phase A: q/k projection -> qk_sb [128, 4, S] ----
            # ft 0,1 = q for head-pair col groups hp0/hp1; ft 2,3 = k.
            # partitions: p = 64*(h%2) + d within each ft tile.
            # Emitted interleaved with the merged pipeline (emit_A below).
            qk_sb = wpool.tile([128, FQK // 128, S], BF16, tag="qk")

            def emit_A(pc):
                # q/k projection for one 256-seq piece (starts on one x DMA)
                for ft in range(4):
                    pt = pop.tile([128, 512], F32, tag=f"po{ft % 2}",
                                  bufs=1, name=f"po{ft % 2}")
                    for kt in range(KT):
                        nc.tensor.matmul(
                            pt[:, :256],
                            wq[:, ft // 2, kt,
                               (ft % 2) * 128:(ft % 2) * 128 + 128],
                            xT[:, pc, kt, :],
                            start=(kt == 0), stop=(kt == KT - 1))
                    nc.scalar.activation(
                        qk_sb[:, ft, pc * 256:(pc + 1) * 256], pt[:, :256],
                        mybir.ActivationFunctionType.Identity,
                        bias=bqk[:, ft:ft + 1])

            # ---- persistent attention tiles ----
            v_sb = wpool.tile([128, S // 128, HPC, HD + 1], BF16, tag="v")
            nc.vector.memset(v_sb[:, :, :, HD:HD + 1], 1.0)
            vpk = wpool.tile([128, nU, HPC, HD + 1], BF16, tag="vpk")

            def issue_vpk(b):
                lo, _ = bands[b]
                a0, sp = lo // 128, lo % 128
                if sp == 0:
                    nc.gpsimd.dma_start(vpk[:, 2 * b:2 * b + 2], v_sb[:, a0:a0 + 2])
                else:
                    nc.gpsimd.dma_start(vpk[:128 - sp, 2 * b:2 * b + 2],
                                        v_sb[sp:, a0:a0 + 2])
                    nc.gpsimd.dma_start(vpk[128 - sp:, 2 * b:2 * b + 2],
                                        v_sb[:sp, a0 + 1:a0 + 3])

            emts = [None] * NBLK
            pvqs = [None] * NBLK
            aqs = [None] * NBLK
            ats = [None] * NBLK
            pos_ = [None] * NBLK
            obts = [None] * NBLK

            # ---- merged pipeline (phase A interleaved per 256-piece) ----
            emit_A(0)
            emit_A(1)
            for s in range(-2, NBLK + 6):

                # v projection chunk s+2 (PE) + bias add (DVE)
                c = s + 2
                if 0 <= c < NBLK:
                    pb = pbp.tile([128, 512], F32, tag="pb", name="pb")
                    for kt in range(KT):
                        nc.tensor.matmul(
                            pb[:, :256],
                            xT[:, c // 2, kt, (c % 2) * 128:(c % 2) * 128 + 128],
                            wq[:, 2, kt, :],
                            start=(kt == 0), stop=(kt == KT - 1))
                    nc.vector.tensor_add(
                        v_sb[:, c, :, :HD],
                        pb[:, :256].rearrange("p (h d) -> p h d", h=HPC),
                        bvb[:])
                for b in vpk_at.get(s, ()):
                    issue_vpk(b)

                # psum->sbuf output copies + DMA for block s-5
                b5 = s - 5
                if 0 <= b5 < NBLK:
                    obt = obpool.tile([128, DIM], BF16, tag="obt")
                    nc.scalar.copy(obt[:, :512], pos_[b5][0][:])
                    nc.sync.dma_start(
                        out_d[b5 * BLK:(b5 + 1) * BLK, :512], obt[:, :512])
                    nc.vector.tensor_copy(obt[:, 512:], pos_[b5][1][:])
                    obts[b5] = obt
                    nc.sync.dma_start(
                        out_d[b5 * BLK:(b5 + 1) * BLK, 512:], obt[:, 512:])

                # scores for block s: psum [128k, hh, 2*iu+hp, q]
                if 0 <= s < NBLK:
                    lo, _ = bands[s]
                    qs = slice(s * BLK, (s + 1) * BLK)
                    pst = pstp.tile([128, 2, HPC, BLK], F32, tag="pst")
                    for iu in range(2):
                        for hp in range(2):
                            for hh in range(2):
                                nc.tensor.matmul(
                                    pst[:, hh, 2 * iu + hp, :],
                                    qk_sb[64 * hh:64 * hh + 64, 2 + hp,
                                          lo + iu * 128: lo + (iu + 1) * 128],
                                    qk_sb[64 * hh:64 * hh + 64, hp, qs],
                                    start=True, stop=True)
                    et = epool.tile([128, 2, HPC, BLK], BF16, tag="et")
                    nc.scalar.activation(
                        et[:], pst[:], mybir.ActivationFunctionType.Exp,
                        scale=float(SCALE))
                    emts[s] = et  # masked in next step (emt reuses slot)

                # mask multiply for block s-1 (DVE + Pool split)
                b1 = s - 1
                if 0 <= b1 < NBLK:
                    et = emts[b1]
                    emt = epool.tile([128, 2, HPC, BLK], BF16, tag="emt")
                    for iu, eng in ((0, nc.vector), (1, nc.gpsimd)):
                        eng.tensor_mul(
                            emt[:, :, 2 * iu:2 * iu + 2, :],
                            et[:, :, 2 * iu:2 * iu + 2, :],
                            mk[:, 2 * b1 + iu, None, None, :]
                            .broadcast_to((128, 2, 2, BLK)))
                    emts[b1] = emt

                # PV transposed for block s-2: out [q, head, 64v + den],
                # then reciprocal + normalize (DVE, same step)
                b2 = s - 2
                if 0 <= b2 < NBLK:
                    emt = emts[b2]
                    pvq = pvqp.tile([128, HPC, 128], F32, tag="pvq")
                    for h in range(HPC):
                        for iu in range(2):
                            nc.tensor.matmul(
                                pvq[:, h, :HD + 1],
                                emt[:, h % 2, 2 * iu + h // 2, :],
                                vpk[:, 2 * b2 + iu, h, :],
                                start=(iu == 0), stop=(iu == 1))
                    rec = rcpool.tile([128, HPC, 1], F32, tag="rec")
                    nc.vector.reciprocal_approx_fast(
                        out=rec[:], in_=pvq[:, :, HD:HD + 1])
                    aq = aqpool.tile([128, HPC, HD], BF16, tag="aq")
                    nc.vector.tensor_mul(
                        aq[:], pvq[:, :, :HD],
                        rec[:].broadcast_to((128, HPC, HD)))
                    aqs[b2] = aq

                # transpose back to [feature, q] for block s-3 (PE + DVE copy)
                b3 = s - 3
                if 0 <= b3 < NBLK:
                    aq = aqs[b3]
                    ptr = ptrp.tile([128, 2, 512], BF16, tag="ptr", name="ptr")
                    for dt in range(2):
                        nc.tensor.transpose(
                            ptr[:, dt, :128],
                            aq[:, 2 * dt:2 * dt + 2, :].rearrange(
                                "p a b -> p (a b)"),
                            ident[:])
                    at = atpool.tile([128, 2, BLK], BF16, tag="at")
                    nc.vector.tensor_copy(at[:], ptr[:, :, :128])
                    ats[b3] = at

                # output projection for block s-4 (PE)
                b4 = s - 4
                if 0 <= b4 < NBLK:
                    at = ats[b4]
                    pts = []
                    for ot in range(2):
                        pt = pop.tile([128, 512], F32, tag=f"po{ot}",
                                      bufs=1, name=f"po{ot}")
                        for dt in range(2):
                            nc.tensor.matmul(
                                pt[:],
                                at[:, dt, :],
                                wo[:, dt, ot * 512:(ot + 1) * 512],
                                start=(dt == 0), stop=(dt == 1))
                        pts.append(pt)
                    pos_[b4] = pts

                # phase-A piece rides the step tail (PE slack filler);
                # scores(s) never queues behind act-gated A matmuls.
                if 0 <= s <= 10 and s % 2 == 0:
                    emit_A(2 + s // 2)

    nc.finalize()
    return nc


def _build_program_generic(bands):
    """Original (slower) program: handles arbitrary band widths."""
    nU = sum(nb for _, nb in bands)
    nb_max = max(nb for _, nb in bands)

    nc = bacc.Bacc("TRN2", target_bir_lowering=False)

    xT_d = nc.dram_tensor("xT", [DIM, S], BF16, kind="ExternalInput")
    wq_d = nc.dram_tensor("wqkvT", [DIM, FQK + HPC * HD], BF16, kind="ExternalInput")
    bqk_d = nc.dram_tensor("bqkp", [128, FQK // 128], F32, kind="ExternalInput")
    bv_d = nc.dram_tensor("bv", [HPC * HD], F32, kind="ExternalInput")
    wo_d = nc.dram_tensor("woT", [HPC * HD, DIM], BF16, kind="ExternalInput")
    # pre-arranged mask layout [128, nU, 2, BLK] (head-pair duplicated), bf16
    mask_d = nc.dram_tensor("maskT", [128, nU, 2, BLK], BF16, kind="ExternalInput")
    out_d = nc.dram_tensor("out_p", [S, DIM], BF16, kind="ExternalOutput")

    with tile.TileContext(nc) as tc:
        with tc.tile_pool(name="const", bufs=1) as cpool, \
             tc.tile_pool(name="work", bufs=1) as wpool, \
             tc.tile_pool(name="epool", bufs=6) as epool, \
             tc.tile_pool(name="spool", bufs=2) as spool, \
             tc.tile_pool(name="dram", bufs=1, space="DRAM") as dpool, \
             tc.tile_pool(name="pp", bufs=2, space="PSUM") as pp, \
             tc.tile_pool(name="ps", bufs=3, space="PSUM") as ps, \
             tc.tile_pool(name="pv", bufs=3, space="PSUM") as pv:

            # ---- constant loads ----
            xT = cpool.tile([128, KT, S], BF16, tag="xT")
            for kt in range(KT):
                nc.sync.dma_start(
                    xT[:, kt, :],
                    xT_d.rearrange("(t p) s -> p t s", p=128)[:, kt, :])
            wq = cpool.tile([128, KT, FQK + HPC * HD], BF16, tag="wq")
            nc.sync.dma_start(wq[:], wq_d.rearrange("(t p) f -> p t f", p=128))
            bqk = cpool.tile([128, FQK // 128], F32, tag="bqk")
            nc.sync.dma_start(bqk[:], bqk_d[:])
            bvb = cpool.tile([128, HPC * HD], F32, tag="bvb")
            nc.sync.dma_start(bvb[:], bv_d[None, :].to_broadcast((128, HPC * HD)))
            wo = cpool.tile([128, 2, DIM], BF16, tag="wo")
            nc.sync.dma_start(wo[:], wo_d.rearrange("(t p) o -> p t o", p=128))

            # ---- phase A: q/k projection -> qk_sb [128, 4, S] (f-major) ----
            qk_sb = wpool.tile([128, FQK // 128, S], BF16, tag="qk")
            for ft in range(FQK // 128):
                for st in range(S // 512):
                    pt = pp.tile([128, 512], F32, tag="pp")
                    for kt in range(KT):
                        nc.tensor.matmul(
                            pt[:],
                            wq[:, kt, ft * 128:(ft + 1) * 128],
                            xT[:, kt, st * 512:(st + 1) * 512],
                            start=(kt == 0), stop=(kt == KT - 1))
                    nc.scalar.activation(
                        qk_sb[:, ft, st * 512:(st + 1) * 512], pt[:],
                        mybir.ActivationFunctionType.Identity,
                        bias=bqk[:, ft:ft + 1])

            # ---- phase B: v projection -> v_sb [128, 16, HPC, 65] ----
            v_sb = wpool.tile([128, S // 128, HPC, HD + 1], BF16, tag="v")
            nc.vector.memset(v_sb[:], 1.0)
            for st in range(S // 128):
                pt = pp.tile([128, 512], F32, tag="pp")
                for kt in range(KT):
                    nc.tensor.matmul(
                        pt[:, :HPC * HD],
                        xT[:, kt, st * 128:(st + 1) * 128],
                        wq[:, kt, FQK:],
                        start=(kt == 0), stop=(kt == KT - 1))
                nc.vector.tensor_add(
                    v_sb[:, st, :, :HD],
                    pt[:, :HPC * HD].rearrange("p (h d) -> p h d", h=HPC),
                    bvb.rearrange("p (h d) -> p h d", h=HPC))

            # ---- attention ----
            stg_un = wpool.tile([128, 2, S], F32, tag="stg")     # unnormalized attnT
            den_dram = dpool.tile([HPC, S], F32)
            den_sb = [wpool.tile([1, S], F32, tag=f"den{h}", name=f"den_sb{h}")
                      for h in range(HPC)]

            def pack_band(dst, dsl, b):
                """band-pack v (+ones cols) via DVE cross-base chunk copies."""
                lo, nb = bands[b]
                a0, r = lo // 128, lo % 128
                if r == 0:
                    nc.vector.tensor_copy(dst[:, dsl], v_sb[:, a0:a0 + nb])
                else:
                    for j in range(4):
                        sp = (r + 32 * j) % 128
                        sa = a0 + (1 if r + 32 * j >= 128 else 0)
                        nc.vector.tensor_copy(
                            dst[32 * j:32 * (j + 1), dsl],
                            v_sb[sp:sp + 32, sa:sa + nb])

            moff = 0
            for b in range(NBLK):
                lo, nb = bands[b]
                qs = slice(b * BLK, (b + 1) * BLK)
                mkb = spool.tile([128, nb_max, 2, BLK], BF16, tag="mkb")
                nc.sync.dma_start(mkb[:, :nb], mask_d[:, moff:moff + nb])
                vpb = spool.tile([128, nb_max, HPC, HD + 1], BF16, tag="vpb")
                pack_band(vpb, slice(0, nb), b)
                for h in range(HPC):
                    hh, hp = h % 2, h // 2
                    pvt = pv.tile([HD + 1, BLK], F32, tag="pv")
                    for iu in range(nb):
                        pst = ps.tile([128, BLK], F32, tag="ps")
                        nc.tensor.matmul(
                            pst[:],
                            qk_sb[64 * hh:64 * hh + 64, 2 + hp,
                                  lo + iu * 128: lo + (iu + 1) * 128],
                            qk_sb[64 * hh:64 * hh + 64, hp, qs],
                            start=True, stop=True)
                        et = epool.tile([128, BLK], BF16, tag="e")
                        nc.scalar.activation(
                            et[:], pst[:], mybir.ActivationFunctionType.Exp,
                            scale=float(SCALE))
                        emt = epool.tile([128, BLK], BF16, tag="em")
                        nc.vector.tensor_mul(emt[:], et[:],
                                             mkb[:, iu, hh, :])
                        nc.tensor.matmul(
                            pvt[:], vpb[:, iu, h, :], emt[:],
                            start=(iu == 0), stop=(iu == nb - 1))
                    nc.scalar.copy(stg_un[64 * hh:64 * hh + 64, hp, qs], pvt[:HD, :])
                    nc.vector.tensor_copy(den_sb[h][0:1, qs], pvt[HD:HD + 1, :])
                moff += nb

            # ---- normalize + output projection, pipelined in s-quarters ----
            rec_dram = dpool.tile([HPC, S], F32)
            denr = wpool.tile([128, HPC, S // 128], F32, tag="denr")
            recr = wpool.tile([128, HPC, S // 128], F32, tag="recr")
            rec_bc = wpool.tile([128, 2, S], F32, tag="denbc")
            attnT = wpool.tile([128, 2, S], BF16, tag="attnT")
            NQ = 4
            SQ = S // NQ
            AQ = SQ // 128
            for q in range(NQ):
                sq = slice(q * SQ, (q + 1) * SQ)
                for h in range(HPC):
                    nc.sync.dma_start(den_dram[h:h + 1, sq], den_sb[h][0:1, sq])
                nc.sync.dma_start(
                    denr[:, :, q * AQ:(q + 1) * AQ],
                    den_dram[:, sq].rearrange("h (p a) -> p h a", p=128))
                nc.vector.reciprocal(recr[:, :, q * AQ:(q + 1) * AQ],
                                     denr[:, :, q * AQ:(q + 1) * AQ])
                nc.sync.dma_start(
                    rec_dram[:, sq].rearrange("h (p a) -> p h a", p=128),
                    recr[:, :, q * AQ:(q + 1) * AQ])
                for dt in range(2):
                    for hh in range(2):
                        h = 2 * dt + hh
                        nc.sync.dma_start(
                            rec_bc[64 * hh:64 * (hh + 1), dt, sq],
                            rec_dram[h:h + 1, sq].to_broadcast((64, SQ)))
                    nc.vector.tensor_mul(
                        attnT[:, dt, sq], stg_un[:, dt, sq], rec_bc[:, dt, sq])
                for st in range(q * (S // 128) // NQ, (q + 1) * (S // 128) // NQ):
                    for ot in range(DIM // 512):
                        po = pp.tile([128, 512], F32, tag="pp")
                        for dt in range(2):
                            nc.tensor.matmul(
                                po[:],
                                attnT[:, dt, st * 128:(st + 1) * 128],
                                wo[:, dt, ot * 512:(ot + 1) * 512],
                                start=(dt == 0), stop=(dt == 1))
                        ob = epool.tile([128, 512], BF16, tag="ob")
                        nc.vector.tensor_copy(ob[:], po[:])
                        nc.sync.dma_start(
                            out_d[st * 128:(st + 1) * 128, ot * 512:(ot + 1) * 512],
                            ob[:])

    nc.finalize()
    return nc


def kernel(x, w_qkv, b_qkv, w_out, b_out, routes):
    global LAST_RESULTS
    x = np.asarray(x, dtype=np.float32)
    w_qkv = np.asarray(w_qkv, dtype=np.float32)
    b_qkv = np.asarray(b_qkv, dtype=np.float32)
    w_out = np.asarray(w_out, dtype=np.float32)
    b_out = np.asarray(b_out, dtype=np.float32)
    routes = np.asarray(routes)

    # --- host: permutation + bands + masks ---
    cantor = _cantor_values(S, DEPTH)
    perm = np.lexsort((np.arange(S), cantor))
    inv_perm = np.empty(S, dtype=np.int64)
    inv_perm[perm] = np.arange(S)
    routes_p = inv_perm[routes.astype(np.int64)[perm]]
    bands = _plan_bands(routes_p)
    fast = all(nb == 2 for _, nb in bands)
    maskT = _build_masks(routes_p, bands, 1 if fast else 2)

    key = (tuple(bands), "v2")
    if key not in _PROGRAM_CACHE:
        _PROGRAM_CACHE[key] = (
            _build_program_fast(bands) if fast else _build_program_generic(bands))
    nc = _PROGRAM_CACHE[key]

    # --- host: per-core inputs ---
    x_p = x[:, perm, :]                                   # [B, S, DIM]
    ident = np.eye(128, dtype=BF16NP)
    in_maps = []
    for c in range(N_CORES):
        b = c // (N_CORES // B)
        hg = c % (N_CORES // B)
        heads = range(hg * HPC, (hg + 1) * HPC)
        # w rows: q heads, k heads, v heads
        rows = ([h * HD + i for h in heads for i in range(HD)]
                + [DIM + h * HD + i for h in heads for i in range(HD)]
                + [2 * DIM + h * HD + i for h in heads for i in range(HD)])
        rows = np.asarray(rows)
        wq_c = np.ascontiguousarray(w_qkv[rows].T).astype(BF16NP)   # [1024, 768]
        # bias packed [128, 4] partition-major (feature = ft*128 + p)
        bqk_c = np.ascontiguousarray(
            b_qkv[rows[:FQK]].reshape(FQK // 128, 128).T).astype(np.float32)
        bv_c = np.ascontiguousarray(b_qkv[rows[FQK:]]).astype(np.float32)
        wo_c = np.ascontiguousarray(
            w_out[:, hg * HPC * HD:(hg + 1) * HPC * HD].T).astype(BF16NP)
        xT_c = np.ascontiguousarray(x_p[b].T).astype(BF16NP)
        if fast:
            # piece-contiguous layouts: [128, piece/group, kt, 256]
            in_maps.append({
                "xTp": np.ascontiguousarray(
                    xT_c.reshape(KT, 128, 8, 256).transpose(1, 2, 0, 3)),
                "wqp": np.ascontiguousarray(
                    wq_c.reshape(KT, 128, 3, 256).transpose(1, 2, 0, 3)),
                "bqkp": bqk_c,
                "bv": bv_c,
                "wop": np.ascontiguousarray(
                    wo_c.reshape(2, 128, DIM).transpose(1, 0, 2)),
                "maskT": maskT,
                "ident": ident,
            })
        else:
            in_maps.append({
                "xT": xT_c,
                "wqkvT": wq_c,
                "bqkp": bqk_c,
                "bv": bv_c,
                "woT": wo_c,
                "maskT": maskT,
            })

    try:
        res = run_bass_kernel_spmd(nc, in_maps, core_ids=list(range(N_CORES)))
    except Exception:
        if os.environ.get("BASS_TRACE"):
            # tracing infra failure — retry without profiling
            os.environ["BASS_NEVER_TRACE"] = "1"
            res = run_bass_kernel_spmd(nc, in_maps, core_ids=list(range(N_CORES)))
        else:
            raise
    LAST_RESULTS = res

    out = np.zeros((B, S, DIM), dtype=np.float32)
    for c in range(N_CORES):
        out[c // (N_CORES // B)] += np.asarray(res.results[c]["out_p"],
                                               dtype=np.float32)
    out += b_out[None, None, :]
    out = out[:, inv_perm, :]    # un-permute rows
    return out
